# Initial kernel scaffold
#
"""MLA attention kernel for Trainium2, sharded over 8 NeuronCores.

Sharding: core = (batch b in {0,1}) x (kv-group g in {0..3}).
Each core handles one batch's 4 query heads + 1 kv head of one group and
produces a partial output projection [T, DIM]; the host sums the 4 group
partials per batch.

On-device layout strategy (per core):
  - All matmul inputs are bf16, accumulation fp32 in PSUM.
  - x is fed pre-transposed (dim-major) so Q/lat projections need no
    on-device transpose of x.
  - Q and K are produced token-major [t, d] for rms-norm + RoPE (free-dim
    reductions), then PE-transposed per 128x128 tile into head-major
    [d, t] for the S = Q.T K matmul.
  - S tiles [tq=128, tk<=512] softmax along free dim; exp+rowsum fused on
    ScalarE (accum_out); P normalized in bf16; P transposed per 128x128
    block on the PE so PV can accumulate O.T = sum_k V[k].T @ P.T[k].
  - O.T [d, t] feeds the output projection directly as lhsT.
"""

import os
from contextlib import ExitStack

import numpy as np
import ml_dtypes

import concourse.bass as bass
import concourse.bacc as bacc
import concourse.tile as tile
from concourse import mybir
from concourse.bass_utils import run_bass_kernel_spmd
from concourse.masks import make_identity

BF16 = np.float16
NH, NKV, HD, RANK, DIM = 16, 4, 128, 512, 2048
B, T = 2, 2048
NT = T // 128          # 16 token tiles
NCH = T // 512         # 4 token chunks
KD = DIM // 128        # 16 contraction tiles over model dim
KR = RANK // 128       # 4 contraction tiles over rank
EPS = 1.1920928955078125e-07
SCALE = 1.0 / float(np.sqrt(HD))
EXP_BIAS = -2.0794415416798357  # -ln(8): keeps exp outputs inside fp16 range; cancels in normalize

F32 = mybir.dt.float32
BF = mybir.dt.float16   # fp16: same PE throughput as bf16, 8x the mantissa
AF = mybir.ActivationFunctionType
AX = mybir.AxisListType


def _rope_tables():
    inv_freq = 1.0 / (10000.0 ** (np.arange(0, HD, 2, dtype=np.float64) / HD))
    t = np.arange(T, dtype=np.float64)
    f = np.outer(t, inv_freq)                      # [T, 64]
    cos, sin = np.cos(f), np.sin(f)
    coscat = np.concatenate([cos, cos], axis=1)    # [T, 128]
    sincat = np.concatenate([sin, -sin], axis=1)   # [T, 128]
    return coscat.astype(np.float32), sincat.astype(np.float32)


def _build_module():
    nc = bacc.Bacc("TRN2", target_bir_lowering=False, debug=False)

    # DRAM tensors are laid out exactly as their SBUF tiles (partition-major)
    # so every load is one fully-contiguous DMA.
    xt = nc.dram_tensor("xt", [NCH, 128, KD, 512], BF, kind="ExternalInput").ap()
    wq = nc.dram_tensor("wq", [128, KD, 512], BF, kind="ExternalInput").ap()
    wdn = nc.dram_tensor("wdn", [128, KD, 512], BF, kind="ExternalInput").ap()
    wkv = nc.dram_tensor("wkv", [128, KR, 256], BF, kind="ExternalInput").ap()
    wpj = nc.dram_tensor("wpj", [128, 4, 2048], BF, kind="ExternalInput").ap()
    cosd = nc.dram_tensor("cosd", [128, NT, 128], BF, kind="ExternalInput").ap()
    sind = nc.dram_tensor("sind", [128, NT, 128], BF, kind="ExternalInput").ap()
    gain = nc.dram_tensor("gain", [128, 4], F32, kind="ExternalInput").ap()
    out = nc.dram_tensor("out", [T, DIM], F32, kind="ExternalOutput").ap()

    with tile.TileContext(nc) as tc:
        with ExitStack() as ctx:
            _emit(ctx, tc, out, xt, wq, wdn, wkv, wpj, cosd, sind, gain)
    nc.compile()
    return nc


def _emit(ctx, tc, out, xt, wq, wdn, wkv, wpj, cosd, sind, gain):
    nc = tc.nc

    persist = ctx.enter_context(tc.tile_pool(name="persist", bufs=1))
    qt_sb = persist.tile([128, 4, T], BF)      # Q.T  [d, (h), t]
    kt_sb = persist.tile([128, T], BF)         # K.T  [d, t]
    v_sb = persist.tile([128, NT, 128], BF)    # V    [t, (tile), d]
    ot_sb = persist.tile([128, 4, T], BF)      # O.T  [d, (h), t]
    trimask_t = persist.tile([128, 128], F32)  # keep where tk <= tq
    ident = persist.tile([128, 128], BF)
    ones_sb = persist.tile([128, 1], BF)
    eps_sb = persist.tile([128, 1], F32)
    gain_sb = persist.tile([128, 4], F32)
    cos_sb = persist.tile([128, NT, 128], BF)
    sin_sb = persist.tile([128, NT, 128], BF)

    make_identity(nc, ident)
    # transposed causal mask for S.T tiles [tk, tq]: keep x<=y, else -1e9
    nc.gpsimd.memset(trimask_t, 0.0)
    nc.gpsimd.affine_select(
        out=trimask_t, in_=trimask_t,
        compare_op=mybir.AluOpType.is_ge, fill=-1e9,
        base=0, pattern=[[1, 128]], channel_multiplier=-1,
    )
    expb_sb = persist.tile([128, 1], F32)
    nc.vector.memset(ones_sb, 1.0)
    nc.vector.memset(eps_sb, EPS)
    nc.vector.memset(expb_sb, EXP_BIAS)

    xtp = ctx.enter_context(tc.tile_pool(name="xtp", bufs=2))
    w1 = ctx.enter_context(tc.tile_pool(name="w1", bufs=1))
    latp = ctx.enter_context(tc.tile_pool(name="latp", bufs=1))
    scp = ctx.enter_context(tc.tile_pool(name="sc1", bufs=3))
    ptp = ctx.enter_context(tc.tile_pool(name="ptp", bufs=6))
    sc2 = ctx.enter_context(tc.tile_pool(name="sc2", bufs=2))
    osb = ctx.enter_context(tc.tile_pool(name="osb", bufs=3))
    mmp = ctx.enter_context(tc.tile_pool(name="mm", bufs=3, space="PSUM"))
    spool = ctx.enter_context(tc.tile_pool(name="sps", bufs=3, space="PSUM"))
    opool = ctx.enter_context(tc.tile_pool(name="ops", bufs=1, space="PSUM"))
    trp = ctx.enter_context(tc.tile_pool(name="trp", bufs=1, space="PSUM"))
    kvp = mmp

    wq_sb = w1.tile([128, KD, 512], BF)
    wdn_sb = w1.tile([128, KD, 512], BF)
    xtc0 = xtp.tile([128, KD, 512], BF, tag="xtc")
    for k in range(KD):
        nc.sync.dma_start(out=wdn_sb[:, k, :], in_=wdn[:, k, :])
        nc.sync.dma_start(out=xtc0[:, k, :], in_=xt[0, :, k, :])
        nc.sync.dma_start(out=wq_sb[:, k, :], in_=wq[:, k, :])
    nc.sync.dma_start(out=cos_sb, in_=cosd)
    nc.sync.dma_start(out=sin_sb, in_=sind)
    nc.sync.dma_start(out=gain_sb, in_=gain)
    wkv_sb = w1.tile([128, KR, 256], BF)
    nc.sync.dma_start(out=wkv_sb, in_=wkv)
    wpj_sb = w1.tile([128, 4, 2048], BF)
    nc.sync.dma_start(out=wpj_sb, in_=wpj)
    lat_sb = latp.tile([128, KR, T], BF)   # lat.T [r, (tile), t]


    def emit_phase1(c, xtc):
            # ---- lat.T tiles for this token chunk
            for m in range(KR):
                ps = mmp.tile([128, 512], F32, tag="mm")
                for k in range(KD):
                    nc.tensor.matmul(
                        ps,
                        lhsT=wdn_sb[:, k, m * 128:(m + 1) * 128],
                        rhs=xtc[:, k, :],
                        start=(k == 0),
                        stop=(k == KD - 1),
                    )
                nc.scalar.copy(lat_sb[:, m, c * 512:(c + 1) * 512], ps)

            # ---- K, V for the 4 token tiles of this chunk
            for i in range(4):
                t = 4 * c + i
                ps = kvp.tile([128, 2, 128], F32, tag="mm")
                for m in range(KR):
                    nc.tensor.matmul(
                        ps.rearrange("p a b -> p (a b)"),
                        lhsT=lat_sb[:, m, t * 128:(t + 1) * 128],
                        rhs=wkv_sb[:, m, :],
                        start=(m == 0),
                        stop=(m == KR - 1),
                    )
                kvf = scp.tile([128, 2, 128], F32, tag="kvf")
                nc.scalar.copy(kvf, ps)
                nc.vector.tensor_copy(v_sb[:, t, :], kvf[:, 1, :])
                sqk = scp.tile([128, 128], F32, tag="sq")
                ssk = scp.tile([128, 1], F32, tag="ssk")
                nc.scalar.activation(
                    out=sqk, in_=kvf[:, 0, :], func=AF.Square, accum_out=ssk
                )
                rstdk = scp.tile([128, 1], F32, tag="rstdk")
                nc.scalar.activation(
                    out=rstdk, in_=ssk, func=AF.Sqrt, bias=eps_sb, scale=1.0 / HD
                )
                rstdk2 = scp.tile([128, 1], F32, tag="rstdk2")
                nc.vector.reciprocal_approx_fast(out=rstdk2, in_=rstdk)
                nc.vector.tensor_scalar_mul(
                    kvf[:, 0, :], in0=kvf[:, 0, :], scalar1=rstdk2
                )
                kn = scp.tile([128, 1, 128], BF, tag="kn")
                _rope(nc, scp, kn, kvf[:, 0:1, :], cos_sb[:, t, :], sin_sb[:, t, :], 1)
                tpk = trp.tile([128, 4, 128], BF, tag="tr")
                nc.tensor.transpose(tpk[:, 0, :], kn[:, 0, :], ident)
                nc.scalar.copy(kt_sb[:, t * 128:(t + 1) * 128], tpk[:, 0, :])

            # ---- Q for the 4 token tiles of this chunk
            for i in range(4):
                t = 4 * c + i
                ps = mmp.tile([128, 4, 128], F32, tag="mm")
                for k in range(KD):
                    nc.tensor.matmul(
                        ps.rearrange("p a b -> p (a b)"),
                        lhsT=xtc[:, k, i * 128:(i + 1) * 128],
                        rhs=wq_sb[:, k, :],
                        start=(k == 0),
                        stop=(k == KD - 1),
                    )
                # free the PSUM slot fast: norm + rope run on an SBUF copy
                qf = scp.tile([128, 4, 128], F32, tag="qf")
                nc.scalar.copy(qf, ps)
                sq = scp.tile([128, 128], F32, tag="sq")
                ss = scp.tile([128, 4], F32, tag="ss")
                for h in range(4):
                    nc.scalar.activation(
                        out=sq, in_=qf[:, h, :], func=AF.Square,
                        accum_out=ss[:, h:h + 1],
                    )
                rstd = scp.tile([128, 4], F32, tag="rstd")
                nc.scalar.activation(
                    out=rstd, in_=ss, func=AF.Sqrt, bias=eps_sb, scale=1.0 / HD
                )
                rstd2 = scp.tile([128, 4], F32, tag="rstd2")
                nc.vector.reciprocal_approx_fast(out=rstd2, in_=rstd)
                nc.vector.tensor_mul(rstd2, rstd2, gain_sb)
                for h in range(4):
                    nc.vector.tensor_scalar_mul(
                        qf[:, h, :], in0=qf[:, h, :], scalar1=rstd2[:, h:h + 1]
                    )
                # rope (all 4 heads per op via broadcast table AP)
                qn = scp.tile([128, 4, 128], BF, tag="qn")
                _rope(nc, scp, qn, qf, cos_sb[:, t, :], sin_sb[:, t, :], 4)
                tpt = trp.tile([128, 4, 128], BF, tag="tr")
                for h in range(4):
                    nc.tensor.transpose(tpt[:, h, :], qn[:, h, :], ident)
                nc.scalar.copy(qt_sb[:, :, t * 128:(t + 1) * 128], tpt)

    def emit_attn_proj(c):
            # ---- attention for this query chunk (S computed transposed)
            for h in range(4):
                po = opool.tile([128, 512], F32, tag="o")
                pacc = sc2.tile([128, 512], BF, tag="pacc")
                last = 4 * c + 3
                for kk in range(4 * c + 4):
                    j = kk - 4 * c
                    x0 = max(0, j) * 128
                    st = spool.tile([128, 512], F32, tag="s")
                    nc.tensor.matmul(
                        st[:, x0:512],
                        lhsT=kt_sb[:, kk * 128:(kk + 1) * 128],
                        rhs=qt_sb[:, h, c * 512 + x0:(c + 1) * 512],
                        start=True,
                        stop=True,
                    )
                    if j >= 0:
                        nc.vector.tensor_add(
                            st[:, x0:x0 + 128], st[:, x0:x0 + 128], trimask_t
                        )
                    pt = ptp.tile([128, 512], BF, tag="pt")
                    nc.scalar.activation(
                        out=pt[:, x0:512], in_=st[:, x0:512],
                        func=AF.Exp, scale=SCALE, bias=expb_sb,
                    )
                    if kk == 0:
                        nc.vector.tensor_copy(pacc, pt)
                    else:
                        nc.vector.tensor_add(
                            pacc[:, x0:512], pacc[:, x0:512], pt[:, x0:512]
                        )
                    nc.tensor.matmul(
                        po[:, x0:512],
                        lhsT=v_sb[:, kk, :],
                        rhs=pt[:, x0:512],
                        start=(kk == 0),
                        stop=(kk == last),
                        skip_group_check=True,
                    )
                rsp = spool.tile([128, 512], F32, tag="s")
                nc.tensor.matmul(rsp[0:1, :], lhsT=ones_sb, rhs=pacc,
                                 start=True, stop=True)
                rs_sb = sc2.tile([1, 512], F32, tag="rsb")
                nc.vector.reciprocal_approx_fast(out=rs_sb, in_=rsp[0:1, :])
                rbc = sc2.tile([128, 512], F32, tag="rbc")
                nc.gpsimd.partition_broadcast(rbc, rs_sb)
                otmp = sc2.tile([128, 512], BF, tag="otmp")
                nc.vector.tensor_copy(otmp, po)
                nc.vector.tensor_mul(ot_sb[:, h, c * 512:(c + 1) * 512], otmp, rbc)

            # ---- output projection for this chunk
            for i in range(4):
                t = 4 * c + i
                for n in range(4):
                    pj = mmp.tile([128, 512], F32, tag="mm")
                    for h in range(4):
                        nc.tensor.matmul(
                            pj,
                            lhsT=ot_sb[:, h, t * 128:(t + 1) * 128],
                            rhs=wpj_sb[:, h, n * 512:(n + 1) * 512],
                            start=(h == 0),
                            stop=(h == 3),
                        )
                    outsb = osb.tile([128, 512], F32, tag="out")
                    nc.vector.tensor_copy(outsb, pj)
                    nc.sync.dma_start(
                        out=out[t * 128:(t + 1) * 128, n * 512:(n + 1) * 512],
                        in_=outsb,
                    )

    for c in range(NCH):
        if c == 0:
            xtc = xtc0
        else:
            xtc = xtp.tile([128, KD, 512], BF, tag="xtc")
            for k in range(KD):
                nc.sync.dma_start(out=xtc[:, k, :], in_=xt[c, :, k, :])
        emit_phase1(c, xtc)
        if c >= 1:
            emit_attn_proj(c - 1)
    emit_attn_proj(NCH - 1)

def _rope(nc, scp, out_t, ps, cos_t, sin_t, nh):
    """out = ps * coscat + swap_halves(ps) * sincat, per head.

    ps: [128, nh, 128] fp32 (PSUM), out_t: [128, nh, 128] bf16,
    cos_t/sin_t: [128, 128] bf16 tables (broadcast over the head dim).
    """
    t1 = scp.tile([128, nh, 128], F32, tag=f"ropea{nh}")
    t2 = scp.tile([128, nh, 128], F32, tag=f"ropeb{nh}")
    cos_b = _bcast_mid(cos_t, nh)
    sin_b = _bcast_mid(sin_t, nh)
    nc.vector.tensor_mul(t1, ps, cos_b)
    nc.vector.tensor_mul(t2, _swap_halves(ps), sin_b)
    nc.vector.tensor_add(out_t, t1, t2)


def _bcast_mid(ap2d, nh):
    """[128, 128] -> [128, nh, 128] with 0-stride on the middle dim."""
    if nh == 1:
        return bass.AP(tensor=ap2d.tensor, offset=ap2d.offset,
                       ap=[ap2d.ap[0], [0, 1], ap2d.ap[1]])
    return bass.AP(tensor=ap2d.tensor, offset=ap2d.offset,
                   ap=[ap2d.ap[0], [0, nh], ap2d.ap[1]])


def _swap_halves(ap3d):
    """[128, nh, 128] -> same shape reading cols [64:128, 0:64] of last dim."""
    last = ap3d.ap[-1]
    step = last[0]
    return bass.AP(tensor=ap3d.tensor, offset=ap3d.offset + 64 * step,
                   ap=list(ap3d.ap[:-1]) + [[-64 * step, 2], [step, 64]])


def _ensure_ntff_hook():
    """Install the axon NTFF profiling hook if the image lacks
    antenv.axon_hooks (needed for trace=True under axon)."""
    try:
        from antenv.axon_hooks import get_axon_ntff_profile_hook  # noqa: F401
        return
    except ImportError:
        pass
    import contextlib
    import ctypes
    import sys
    import types

    mod = types.ModuleType("antenv.axon_hooks")
    _state = {"hook": None}
    mod.set_axon_ntff_profile_hook = lambda h: _state.update(hook=h)
    mod.get_axon_ntff_profile_hook = lambda: _state["hook"]
    import antenv

    sys.modules["antenv.axon_hooks"] = mod
    antenv.axon_hooks = mod

    so_path = "/opt/axon/libaxon_pjrt.so"
    if not os.path.exists(so_path):
        return
    lib = ctypes.CDLL(so_path)
    if not hasattr(lib, "axon_start_nrt_profile"):
        return
    lib.axon_start_nrt_profile.argtypes = [
        ctypes.POINTER(ctypes.c_int64),
        ctypes.c_size_t,
    ]
    lib.axon_start_nrt_profile.restype = ctypes.c_int64
    lib.axon_stop_nrt_profile.argtypes = [ctypes.c_char_p]
    lib.axon_stop_nrt_profile.restype = ctypes.c_int64

    @contextlib.contextmanager
    def _hook(output_dir, device_ids):
        import jax

        jax.devices()
        if device_ids:
            ids = (ctypes.c_int64 * len(device_ids))(*device_ids)
            rc = lib.axon_start_nrt_profile(ids, len(device_ids))
        else:
            rc = lib.axon_start_nrt_profile(None, 0)
        if rc != 0:
            raise RuntimeError(f"axon_start_nrt_profile rc={rc}")
        try:
            yield
        finally:
            n = lib.axon_stop_nrt_profile(str(output_dir).encode())
            if n < 0:
                raise RuntimeError(f"axon_stop_nrt_profile rc={n}")
            print(f"profile: {n} file(s) written to {output_dir}")

    mod.set_axon_ntff_profile_hook(_hook)


_NC_CACHE = None


def _get_module():
    global _NC_CACHE
    if _NC_CACHE is None:
        _NC_CACHE = _build_module()
    return _NC_CACHE


def _prep_core_inputs(x, Wq, Wdown, Wkup, Wvup, Wproj, q_gain, b, g):
    coscat, sincat = _rope_tables()
    xb = x[b].astype(BF16)                                   # [T, DIM]
    xt = np.ascontiguousarray(
        xb.reshape(NCH, 512, KD, 128).transpose(0, 3, 2, 1)
    )                                                        # [4,128,16,512]
    wqg = Wq[g * 512:(g + 1) * 512].astype(BF16)             # [512, 2048]
    wq = np.ascontiguousarray(wqg.reshape(512, KD, 128).transpose(2, 1, 0))
    wdn = np.ascontiguousarray(
        Wdown.astype(BF16).reshape(512, KD, 128).transpose(2, 1, 0)
    )
    wkug = Wkup[g * 128:(g + 1) * 128].astype(BF16)          # [128, 512]
    wku = wkug.reshape(128, KR, 128).transpose(2, 1, 0)
    wvug = Wvup[g * 128:(g + 1) * 128].astype(BF16)
    wvu = wvug.reshape(128, KR, 128).transpose(2, 1, 0)
    wkv = np.ascontiguousarray(np.concatenate([wku, wvu], axis=2))
    wpg = Wproj[:, g * 512:(g + 1) * 512].astype(BF16)       # [2048, 512]
    wpj = np.ascontiguousarray(wpg.reshape(2048, 4, 128).transpose(2, 1, 0))
    cos = np.ascontiguousarray(
        coscat.astype(BF16).reshape(NT, 128, 128).transpose(1, 0, 2)
    )
    sin = np.ascontiguousarray(
        sincat.astype(BF16).reshape(NT, 128, 128).transpose(1, 0, 2)
    )
    gain = np.ascontiguousarray(
        np.broadcast_to(q_gain[g * 4:(g + 1) * 4].astype(np.float32), (128, 4))
    )
    return {
        "xt": xt, "wq": wq, "wdn": wdn, "wkv": wkv,
        "wpj": wpj, "cosd": cos, "sind": sin, "gain": gain,
    }


def kernel(x, Wq, Wdown, Wkup, Wvup, Wproj, q_gain, _trace=False):
    x = np.asarray(x, dtype=np.float32)
    nc = _get_module()
    in_maps = []
    for core in range(8):
        b, g = divmod(core, 4)
        in_maps.append(
            _prep_core_inputs(x, np.asarray(Wq), np.asarray(Wdown),
                              np.asarray(Wkup), np.asarray(Wvup),
                              np.asarray(Wproj), np.asarray(q_gain), b, g)
        )
    if _trace:
        _ensure_ntff_hook()
    res = run_bass_kernel_spmd(nc, in_maps, core_ids=list(range(8)),
                               trace=_trace)
    outs = [r["out"] for r in res.results]
    y = np.empty((B, T, DIM), dtype=np.float32)
    for b in range(B):
        y[b] = outs[4 * b + 0] + outs[4 * b + 1] + outs[4 * b + 2] + outs[4 * b + 3]
    kernel._last_results = res
    return y



# revision 23
# speedup vs baseline: 1.3251x; 1.3251x over previous
"""MLA attention kernel for Trainium2, sharded over 8 NeuronCores.

Sharding: core = (batch b in {0,1}) x (kv-group g in {0..3}).
Each core handles one batch's 4 query heads + 1 kv head of one group and
produces a partial output projection [T, DIM]; the host sums the 4 group
partials per batch.

Strategy (per core):
  - Wdown folded into Wkup/Wvup on the host (exact), K/V projected
    straight from x.
  - Pass order: all K/V tiles first, then q(3); attn(3) -- whose
    ScalarE exp stream is filled on TensorE by q(2..0) emitted after
    it -- then proj(3), and attn(c)+proj(c) for c=2..0 so each chunk's
    projection fills the next (smaller) attention block's exp stalls.
  - PSUM banks drain fast (single copy) to keep the matmul pipeline
    fed; sum-of-squares on VectorE (tensor_mul + reduce_sum); ONE
    chunk-level [128,16] Sqrt (act-table switches cost ~1.3us, so
    ScalarE stays on Exp+Copy during attention).
  - S tiles [tk=128, tq<=512]: exp on ScalarE; diag blocks masked
    post-exp by a 0/1 triangle multiply on GpSimd; P accumulated into
    pacc (VectorE) for the softmax denominator; PV accumulates O.T.
  - Rowsum via all-ones [128,128] lhsT matmul (broadcast across
    partitions), reciprocal + one fused scalar_tensor_tensor normalize.
  - Partial outputs written fp16, summed on the host in fp32.
"""

import os
from contextlib import ExitStack

import numpy as np
import ml_dtypes

import concourse.bass as bass
import concourse.bacc as bacc
import concourse.tile as tile
from concourse import mybir
from concourse.bass_utils import run_bass_kernel_spmd
from concourse.masks import make_identity

BF16 = np.float16
NH, NKV, HD, RANK, DIM = 16, 4, 128, 512, 2048
B, T = 2, 2048
NT = T // 128          # 16 token tiles
NCH = T // 512         # 4 token chunks
KD = DIM // 128        # 16 contraction tiles over model dim
EPS = 1.1920928955078125e-07
SCALE = 1.0 / float(np.sqrt(HD))
EXP_BIAS = -2.0794415416798357  # -ln(8): keeps exp outputs inside fp16 range; cancels in normalize

F32 = mybir.dt.float32
BF = mybir.dt.float16   # fp16: same PE throughput as bf16, 8x the mantissa
AF = mybir.ActivationFunctionType
ALU = mybir.AluOpType


def _rope_tables():
    inv_freq = 1.0 / (10000.0 ** (np.arange(0, HD, 2, dtype=np.float64) / HD))
    t = np.arange(T, dtype=np.float64)
    f = np.outer(t, inv_freq)                      # [T, 64]
    cos, sin = np.cos(f), np.sin(f)
    coscat = np.concatenate([cos, cos], axis=1)    # [T, 128]
    sincat = np.concatenate([sin, -sin], axis=1)   # [T, 128]
    return coscat.astype(np.float32), sincat.astype(np.float32)


def _build_module():
    nc = bacc.Bacc("TRN2", target_bir_lowering=False, debug=False)

    # DRAM tensors are laid out exactly as their SBUF tiles (partition-major)
    # so every load is one fully-contiguous DMA.
    xt = nc.dram_tensor("xt", [NCH, 128, KD, 512], BF, kind="ExternalInput").ap()
    wq = nc.dram_tensor("wq", [128, KD, 512], BF, kind="ExternalInput").ap()
    wkv = nc.dram_tensor("wkv", [128, KD, 256], BF, kind="ExternalInput").ap()
    wpj = nc.dram_tensor("wpj", [128, 4, 2048], BF, kind="ExternalInput").ap()
    cosd = nc.dram_tensor("cosd", [128, NT, 128], BF, kind="ExternalInput").ap()
    sind = nc.dram_tensor("sind", [128, NT, 128], BF, kind="ExternalInput").ap()
    gain = nc.dram_tensor("gain", [128, 4], F32, kind="ExternalInput").ap()
    out = nc.dram_tensor("out", [T, DIM], BF, kind="ExternalOutput").ap()

    with tile.TileContext(nc) as tc:
        with ExitStack() as ctx:
            _emit(ctx, tc, out, xt, wq, wkv, wpj, cosd, sind, gain)
    nc.compile()
    return nc


def _emit(ctx, tc, out, xt, wq, wkv, wpj, cosd, sind, gain):
    nc = tc.nc

    persist = ctx.enter_context(tc.tile_pool(name="persist", bufs=1))
    qt_sb = persist.tile([128, 4, T], BF)      # Q.T  [d, (h), t]
    kt_sb = persist.tile([128, T], BF)         # K.T  [d, t]
    v_sb = persist.tile([128, NT, 128], BF)    # V    [t, (tile), d]
    ot_sb = persist.tile([128, 4, T], BF)      # O.T  [d, (h), t]
    tri01 = persist.tile([128, 128], BF)       # 1 where tk <= tq else 0
    ident = persist.tile([128, 128], BF)
    ones_sb = persist.tile([128, 128], BF)     # all-ones lhsT for rowsum bcast
    eps_sb = persist.tile([128, 1], F32)
    gain_sb = persist.tile([128, 4], F32)
    cos_sb = persist.tile([128, NT, 128], BF)
    sin_sb = persist.tile([128, NT, 128], BF)
    expb_sb = persist.tile([128, 1], F32)

    make_identity(nc, ident)
    nc.vector.memset(ones_sb, 1.0)
    # 0/1 transposed-causal triangle for P.T diag blocks [tk, tq]:
    # keep (1.0) where tq >= tk, else 0.
    nc.gpsimd.memset(tri01, 1.0)
    nc.gpsimd.affine_select(
        out=tri01, in_=tri01,
        compare_op=ALU.is_ge, fill=0.0,
        base=0, pattern=[[1, 128]], channel_multiplier=-1,
    )
    nc.vector.memset(eps_sb, EPS)
    nc.vector.memset(expb_sb, EXP_BIAS)

    xtp = ctx.enter_context(tc.tile_pool(name="xtp", bufs=4))
    w1 = ctx.enter_context(tc.tile_pool(name="w1", bufs=1))
    scp = ctx.enter_context(tc.tile_pool(name="scp", bufs=3))
    qrp = ctx.enter_context(tc.tile_pool(name="qrp", bufs=5))
    ptp = ctx.enter_context(tc.tile_pool(name="ptp", bufs=6))
    sc2 = ctx.enter_context(tc.tile_pool(name="sc2", bufs=2))
    osb = ctx.enter_context(tc.tile_pool(name="osb", bufs=3))
    mmp = ctx.enter_context(tc.tile_pool(name="mm", bufs=3, space="PSUM"))
    spool = ctx.enter_context(tc.tile_pool(name="sps", bufs=2, space="PSUM"))
    opool = ctx.enter_context(tc.tile_pool(name="ops", bufs=2, space="PSUM"))
    trp = ctx.enter_context(tc.tile_pool(name="trp", bufs=1, space="PSUM"))

    # ---- all input DMAs up front (16 queues; overlap the KV pass).
    # wkv + chunk-0 x slices go first, split per-k, so the first KV
    # accumulation can start as soon as its k-slices land.
    wq_sb = w1.tile([128, KD, 512], BF)
    wkv_sb = w1.tile([128, KD, 256], BF)
    xtcs = []
    for _c in range(NCH):
        xtc_c = xtp.tile([128, KD, 512], BF, tag="xtc")
        xtcs.append(xtc_c)
    for k in range(KD):
        nc.sync.dma_start(out=wkv_sb[:, k, :], in_=wkv[:, k, :])
        nc.sync.dma_start(out=xtcs[0][:, k, :], in_=xt[0, :, k, :])
    nc.sync.dma_start(out=cos_sb, in_=cosd)
    nc.sync.dma_start(out=sin_sb, in_=sind)
    nc.sync.dma_start(out=gain_sb, in_=gain)
    for c in range(1, NCH):
        for k in range(KD):
            nc.sync.dma_start(out=xtcs[c][:, k, :], in_=xt[c, :, k, :])
    for k in range(KD):
        nc.sync.dma_start(out=wq_sb[:, k, :], in_=wq[:, k, :])
    wpj_sb = w1.tile([128, 4, 2048], BF)
    nc.sync.dma_start(out=wpj_sb, in_=wpj)

    def emit_kv(t):
        c, i = divmod(t, 4)
        ps = mmp.tile([128, 2, 128], F32, tag="mm")
        for k in range(KD):
            nc.tensor.matmul(
                ps.rearrange("p a b -> p (a b)"),
                lhsT=xtcs[c][:, k, i * 128:(i + 1) * 128],
                rhs=wkv_sb[:, k, :],
                start=(k == 0),
                stop=(k == KD - 1),
            )
        # fast PSUM drain: K raw + V, one copy each
        kraw = scp.tile([128, 128], BF, tag="kraw")
        nc.scalar.copy(out=kraw, in_=ps[:, 0, :])
        nc.scalar.copy(out=v_sb[:, t, :], in_=ps[:, 1, :])
        sqk = scp.tile([128, 128], BF, tag="sqk")
        ssk = scp.tile([128, 1], F32, tag="ssk")
        nc.vector.tensor_mul(sqk, kraw, kraw)
        nc.vector.reduce_sum(out=ssk, in_=sqk, axis=mybir.AxisListType.X)
        rstdk = scp.tile([128, 1], F32, tag="rstdk")
        nc.scalar.activation(out=rstdk, in_=ssk, func=AF.Sqrt,
                             bias=eps_sb, scale=1.0 / HD)
        rstdk2 = scp.tile([128, 1], F32, tag="rstdk2")
        nc.vector.reciprocal_approx_fast(out=rstdk2, in_=rstdk)
        # normalize in place, then rope
        nc.vector.tensor_scalar_mul(kraw, in0=kraw, scalar1=rstdk2)
        t1 = scp.tile([128, 128], BF, tag="ropea")
        t2 = scp.tile([128, 128], BF, tag="ropeb")
        nc.vector.tensor_mul(t1, kraw, cos_sb[:, t, :])
        nc.vector.tensor_mul(t2, _swap2(kraw), sin_sb[:, t, :])
        kn = scp.tile([128, 128], BF, tag="kn")
        nc.vector.tensor_add(kn, t1, t2)
        tpk = trp.tile([128, 4, 128], BF, tag="tr")
        nc.tensor.transpose(tpk[:, 0, :], kn, ident)
        nc.scalar.copy(out=kt_sb[:, t * 128:(t + 1) * 128], in_=tpk[:, 0, :])

    def emit_q(c):
        # phase A: projections + sum-of-squares for all 4 tiles, then ONE
        # chunk-level Sqrt -- keeps ScalarE's act-table on Exp during the
        # overlapped attention stream (each function switch costs ~1.3us).
        qraws = []
        ssc = scp.tile([128, 16], F32, tag="ssc")
        for i in range(4):
            ps = mmp.tile([128, 4, 128], F32, tag="mm")
            for k in range(KD):
                nc.tensor.matmul(
                    ps.rearrange("p a b -> p (a b)"),
                    lhsT=xtcs[c][:, k, i * 128:(i + 1) * 128],
                    rhs=wq_sb[:, k, :],
                    start=(k == 0),
                    stop=(k == KD - 1),
                )
            qraw = qrp.tile([128, 4, 128], BF, tag="qraw")
            qraws.append(qraw)
            nc.scalar.copy(out=qraw, in_=ps)
            sq4 = scp.tile([128, 4, 128], BF, tag="sq4")
            nc.vector.tensor_mul(sq4, qraw, qraw)
            nc.vector.reduce_sum(out=ssc[:, i * 4:(i + 1) * 4], in_=sq4,
                                 axis=mybir.AxisListType.X)
        rstdc = scp.tile([128, 16], F32, tag="rstdc")
        nc.scalar.activation(out=rstdc, in_=ssc, func=AF.Sqrt,
                             bias=eps_sb, scale=1.0 / HD)
        rstd2c = scp.tile([128, 16], F32, tag="rstd2c")
        nc.vector.reciprocal_approx_fast(out=rstd2c, in_=rstdc)
        nc.vector.tensor_mul(rstd2c, rstd2c, _bcast_mid(gain_sb, 4))
        # phase B: normalize, rope, transpose per tile
        for i in range(4):
            t = 4 * c + i
            qraw = qraws[i]
            for h in range(4):
                nc.vector.tensor_scalar_mul(
                    qraw[:, h, :], in0=qraw[:, h, :],
                    scalar1=rstd2c[:, i * 4 + h:i * 4 + h + 1],
                )
            qn = scp.tile([128, 4, 128], BF, tag="qn")
            t1 = scp.tile([128, 4, 128], BF, tag="ropea4")
            t2 = scp.tile([128, 4, 128], BF, tag="ropeb4")
            nc.vector.tensor_mul(t1, qraw, _bcast_mid(cos_sb[:, t, :], 4))
            nc.vector.tensor_mul(t2, _swap_halves(qraw),
                                 _bcast_mid(sin_sb[:, t, :], 4))
            nc.vector.tensor_add(qn, t1, t2)
            tpt = trp.tile([128, 4, 128], BF, tag="tr")
            for h in range(4):
                nc.tensor.transpose(tpt[:, h, :], qn[:, h, :], ident)
            nc.scalar.copy(out=qt_sb[:, :, t * 128:(t + 1) * 128], in_=tpt)

    def emit_attn(c, fillers=None):
        fillers = list(fillers) if fillers else []
        nsteps = (4 * c + 4) * 4
        stride = max(1, nsteps // len(fillers)) if fillers else 0
        step = 0
        for h in range(4):
            po = opool.tile([128, 512], F32, tag="o")
            pacc = sc2.tile([128, 512], BF, tag="pacc")
            last = 4 * c + 3
            for kk in range(4 * c + 4):
                j = kk - 4 * c
                x0 = max(0, j) * 128
                st = spool.tile([128, 512], F32, tag="s")
                nc.tensor.matmul(
                    st[:, x0:512],
                    lhsT=kt_sb[:, kk * 128:(kk + 1) * 128],
                    rhs=qt_sb[:, h, c * 512 + x0:(c + 1) * 512],
                    start=True,
                    stop=True,
                )
                pt = ptp.tile([128, 512], BF, tag="pt")
                nc.scalar.activation(
                    out=pt[:, x0:512], in_=st[:, x0:512],
                    func=AF.Exp, scale=SCALE, bias=expb_sb,
                )
                if j >= 0:
                    # zero the above-diagonal of the diag block post-exp
                    nc.gpsimd.tensor_mul(
                        pt[:, x0:x0 + 128], pt[:, x0:x0 + 128], tri01
                    )
                if kk == 0:
                    nc.vector.tensor_copy(pacc, pt)
                else:
                    nc.vector.tensor_add(
                        pacc[:, x0:512], pacc[:, x0:512], pt[:, x0:512]
                    )
                nc.tensor.matmul(
                    po[:, x0:512],
                    lhsT=v_sb[:, kk, :],
                    rhs=pt[:, x0:512],
                    start=(kk == 0),
                    stop=(kk == last),
                    skip_group_check=True,
                )
                # feed one previous-chunk projection group into the PE
                # stream as filler for the exp-gated PV dependency
                step += 1
                if fillers and stride and step % stride == 0:
                    fillers.pop(0)()
            # rowsum broadcast to all partitions via all-ones lhsT
            rsp = spool.tile([128, 512], F32, tag="s")
            nc.tensor.matmul(rsp, lhsT=ones_sb, rhs=pacc,
                             start=True, stop=True)
            rrec = sc2.tile([128, 512], F32, tag="rrec")
            nc.vector.reciprocal_approx_fast(out=rrec, in_=rsp)
            # fused normalize drain: ot = (po * 1.0) * rrec
            nc.vector.scalar_tensor_tensor(
                out=ot_sb[:, h, c * 512:(c + 1) * 512],
                in0=po, scalar=1.0, in1=rrec,
                op0=ALU.mult, op1=ALU.mult,
            )
        for f in fillers:
            f()

    def proj_groups(c):
        fns = []
        for i in range(4):
            for n in range(4):
                def f(i=i, n=n):
                    t = 4 * c + i
                    pj = mmp.tile([128, 512], F32, tag="mm")
                    for h in range(4):
                        nc.tensor.matmul(
                            pj,
                            lhsT=ot_sb[:, h, t * 128:(t + 1) * 128],
                            rhs=wpj_sb[:, h, n * 512:(n + 1) * 512],
                            start=(h == 0),
                            stop=(h == 3),
                        )
                    outsb = osb.tile([128, 512], BF, tag="out")
                    if n % 2 == 0:
                        nc.vector.tensor_copy(outsb, pj)
                    else:
                        nc.scalar.copy(out=outsb, in_=pj)
                    nc.sync.dma_start(
                        out=out[t * 128:(t + 1) * 128,
                                n * 512:(n + 1) * 512],
                        in_=outsb,
                    )
                fns.append(f)
        return fns

    def emit_proj(c):
        for f in proj_groups(c):
            f()

    for t in range(NT):
        emit_kv(t)
    # attention is ScalarE(exp)-paced: the largest chunk's attention is
    # emitted right after its Q pass so the remaining Q projections fill
    # TensorE during its exp stream; after that, each chunk's projection
    # fills the next (smaller) chunk's attention.
    emit_q(NCH - 1)
    emit_attn(NCH - 1)
    for c in range(NCH - 2, -1, -1):
        emit_q(c)
    for c in range(NCH - 2, -1, -1):
        emit_attn(c, fillers=proj_groups(c + 1))
    emit_proj(0)


def _swap2(ap2d):
    """[128, 128] -> same free size reading cols [64:128, 0:64]."""
    step = ap2d.ap[-1][0]
    return bass.AP(tensor=ap2d.tensor, offset=ap2d.offset + 64 * step,
                   ap=[ap2d.ap[0], [-64 * step, 2], [step, 64]])


def _bcast_mid(ap2d, nh):
    """[128, 128] -> [128, nh, 128] with 0-stride on the middle dim."""
    return bass.AP(tensor=ap2d.tensor, offset=ap2d.offset,
                   ap=[ap2d.ap[0], [0, nh], ap2d.ap[1]])


def _swap_halves(ap3d):
    """[128, nh, 128] -> same shape reading cols [64:128, 0:64] of last dim."""
    last = ap3d.ap[-1]
    step = last[0]
    return bass.AP(tensor=ap3d.tensor, offset=ap3d.offset + 64 * step,
                   ap=list(ap3d.ap[:-1]) + [[-64 * step, 2], [step, 64]])


def _ensure_ntff_hook():
    """Install the axon NTFF profiling hook if the image lacks
    antenv.axon_hooks (needed for trace=True under axon)."""
    try:
        from antenv.axon_hooks import get_axon_ntff_profile_hook  # noqa: F401
        return
    except ImportError:
        pass
    import contextlib
    import ctypes
    import sys
    import types

    mod = types.ModuleType("antenv.axon_hooks")
    _state = {"hook": None}
    mod.set_axon_ntff_profile_hook = lambda h: _state.update(hook=h)
    mod.get_axon_ntff_profile_hook = lambda: _state["hook"]
    import antenv

    sys.modules["antenv.axon_hooks"] = mod
    antenv.axon_hooks = mod

    so_path = "/opt/axon/libaxon_pjrt.so"
    if not os.path.exists(so_path):
        return
    lib = ctypes.CDLL(so_path)
    if not hasattr(lib, "axon_start_nrt_profile"):
        return
    lib.axon_start_nrt_profile.argtypes = [
        ctypes.POINTER(ctypes.c_int64),
        ctypes.c_size_t,
    ]
    lib.axon_start_nrt_profile.restype = ctypes.c_int64
    lib.axon_stop_nrt_profile.argtypes = [ctypes.c_char_p]
    lib.axon_stop_nrt_profile.restype = ctypes.c_int64

    @contextlib.contextmanager
    def _hook(output_dir, device_ids):
        import jax

        jax.devices()
        if device_ids:
            ids = (ctypes.c_int64 * len(device_ids))(*device_ids)
            rc = lib.axon_start_nrt_profile(ids, len(device_ids))
        else:
            rc = lib.axon_start_nrt_profile(None, 0)
        if rc != 0:
            raise RuntimeError(f"axon_start_nrt_profile rc={rc}")
        try:
            yield
        finally:
            n = lib.axon_stop_nrt_profile(str(output_dir).encode())
            if n < 0:
                raise RuntimeError(f"axon_stop_nrt_profile rc={n}")
            print(f"profile: {n} file(s) written to {output_dir}")

    mod.set_axon_ntff_profile_hook(_hook)


_NC_CACHE = None


def _get_module():
    global _NC_CACHE
    if _NC_CACHE is None:
        _NC_CACHE = _build_module()
    return _NC_CACHE


def _prep_core_inputs(x, Wq, Wdown, Wkup, Wvup, Wproj, q_gain, b, g):
    coscat, sincat = _rope_tables()
    xb = x[b].astype(BF16)                                   # [T, DIM]
    xt = np.ascontiguousarray(
        xb.reshape(NCH, 512, KD, 128).transpose(0, 3, 2, 1)
    )                                                        # [4,128,16,512]
    wqg = Wq[g * 512:(g + 1) * 512].astype(BF16)             # [512, 2048]
    wq = np.ascontiguousarray(wqg.reshape(512, KD, 128).transpose(2, 1, 0))
    # fold Wdown into the kv up-projections (exact -- linear chain)
    wkg = (Wkup[g * 128:(g + 1) * 128].astype(np.float32)
           @ Wdown.astype(np.float32))                       # [128, 2048]
    wvg = (Wvup[g * 128:(g + 1) * 128].astype(np.float32)
           @ Wdown.astype(np.float32))                       # [128, 2048]
    wcat = np.concatenate([wkg, wvg], axis=0).astype(BF16)   # [256, 2048]
    wkv = np.ascontiguousarray(wcat.reshape(256, KD, 128).transpose(2, 1, 0))
    wpg = Wproj[:, g * 512:(g + 1) * 512].astype(BF16)       # [2048, 512]
    wpj = np.ascontiguousarray(wpg.reshape(2048, 4, 128).transpose(2, 1, 0))
    cos = np.ascontiguousarray(
        coscat.astype(BF16).reshape(NT, 128, 128).transpose(1, 0, 2)
    )
    sin = np.ascontiguousarray(
        sincat.astype(BF16).reshape(NT, 128, 128).transpose(1, 0, 2)
    )
    gain = np.ascontiguousarray(
        np.broadcast_to(q_gain[g * 4:(g + 1) * 4].astype(np.float32), (128, 4))
    )
    return {
        "xt": xt, "wq": wq, "wkv": wkv,
        "wpj": wpj, "cosd": cos, "sind": sin, "gain": gain,
    }


def kernel(x, Wq, Wdown, Wkup, Wvup, Wproj, q_gain, _trace=False):
    x = np.asarray(x, dtype=np.float32)
    nc = _get_module()
    in_maps = []
    for core in range(8):
        b, g = divmod(core, 4)
        in_maps.append(
            _prep_core_inputs(x, np.asarray(Wq), np.asarray(Wdown),
                              np.asarray(Wkup), np.asarray(Wvup),
                              np.asarray(Wproj), np.asarray(q_gain), b, g)
        )
    if _trace:
        _ensure_ntff_hook()
    res = run_bass_kernel_spmd(nc, in_maps, core_ids=list(range(8)),
                               trace=_trace)
    outs = [np.asarray(r["out"], dtype=np.float32) for r in res.results]
    y = np.empty((B, T, DIM), dtype=np.float32)
    for b in range(B):
        y[b] = outs[4 * b + 0] + outs[4 * b + 1] + outs[4 * b + 2] + outs[4 * b + 3]
    kernel._last_results = res
    return y
